# revision 30
# baseline (speedup 1.0000x reference)
"""TGCN (3-step GRU over GCN message passing) on 8 Trainium2 NeuronCores.

Strategy (dst-sharded message passing):
- Host relabels nodes (max-pool over nodes is permutation invariant) with a
  degree-balanced LPT assignment into 8 cores x 98 windows x 128 slots.
- Associativity: gcn(x@lin1) = (Anorm @ (dinv*x)) @ lin1 ... with lin1 and
  the conv weight folded into the gate projections on the host. The gather
  table is therefore dinv*x — pure host data, staged node-major in 4
  contiguous chunks of 25088 rows (int16-indexable). No phase A on device.
- Per (7-window group, src chunk): dma_gather (rotating over 4 SWDGE queues,
  with a deep descriptor-ring carveout so rings drain in parallel with
  generation) fetches per-edge source rows; 0/1 selection matrices built
  with iota+is_equal route each 128-edge block into the group's PSUM
  accumulator via the PE (scatter-add as matmul, gathered block stationary
  so the accumulator is feature-major - no transposes anywhere).
- Self-loops skip the gather: the feature-major dinv*x slice of the core's
  own nodes is DMA'd and added on the vector engine.
- GRU gates as 512-wide fp16 matmuls feature-major, conv+lin1 fused into
  the gate weights (biases are all zero); H stays resident in SBUF.
- Final: per-feature max over the core's nodes, AllReduce-max across cores,
  then the 128x10 output projection (identical on every core).
"""
import sys

sys.path.insert(0, "/opt/trn_rl_repo")

import numpy as np

import concourse.bass as bass
import concourse.mybir as mybir
import concourse.tile as tile
import concourse.bacc as bacc
from concourse.bass import broadcast_tensor_aps
from concourse.bass_utils import run_bass_kernel_spmd

F16 = mybir.dt.float16
F32 = mybir.dt.float32
I16 = mybir.dt.int16
I32 = mybir.dt.int32

N = 100000
E = 1600000
DIN = 128
DH = 128
DOUT = 10
P = 128
NCORE = 8
NW = 98               # windows (128-slot dst tiles) per core
SPC = NW * P          # 12544 slots per core
NSLOT = NCORE * SPC   # 100352
NT = NSLOT // P       # 784 global tiles
REAL_PC = 12500       # real nodes per core; pads at slots [12500, 12544)
CHN = 4               # source chunks: contiguous tile ranges of 196 tiles
CHTILES = NT // CHN   # 196
CHSZ = CHTILES * P    # 25088 rows per chunk (< 32768: int16-safe)
WGS = 7               # windows per gather group
NWG = NW // WGS       # 14 groups
TS = 3


def _preprocess(inputs):
    """Numpy-only host prep: node relabeling, edge sharding, input staging."""
    for b in ("lin1_b", "convb_z", "convb_r", "convb_h",
              "linb_z", "linb_r", "linb_h", "lin2_b"):
        assert np.abs(np.asarray(inputs[b])).max() == 0.0, f"{b} nonzero"

    import heapq

    edges = [np.asarray(inputs[f"edge{t}"]).astype(np.int64) for t in range(TS)]
    deg3 = np.zeros(N, np.int64)
    for t in range(TS):
        deg3 += np.bincount(edges[t][1], minlength=N)
    w_nodes = deg3 + 3

    order = np.argsort(-w_nodes, kind="stable")
    nbins = NCORE * NW
    cap = np.full(nbins, P, np.int32)
    cap[NW - 1 :: NW] = REAL_PC - (NW - 1) * P  # 84 real slots in last window
    heap = [(0, b) for b in range(nbins)]
    heapq.heapify(heap)
    bin_count = np.zeros(nbins, np.int32)
    bin_load = np.zeros(nbins, np.int64)
    assign_bin = np.empty(N, np.int32)
    slot_in_bin = np.empty(N, np.int32)
    for n in order:
        load, b = heapq.heappop(heap)
        assign_bin[n] = b
        slot_in_bin[n] = bin_count[b]
        bin_count[b] += 1
        bin_load[b] += w_nodes[n]
        if bin_count[b] < cap[b]:
            heapq.heappush(heap, (bin_load[b], b))
    core_of = assign_bin // NW
    w_of = assign_bin % NW
    gslot = (core_of * SPC + w_of * P + slot_in_bin).astype(np.int64)

    # degrees (with +1 self loop); pads get 1.0
    dinv = np.empty((TS, NSLOT), np.float32)
    for t in range(TS):
        dd = np.bincount(gslot[edges[t][1]], minlength=NSLOT).astype(np.float64)
        dd += 1.0  # self loops (pads harmlessly get deg 1: their rows are 0)
        dinv[t] = (1.0 / np.sqrt(dd)).astype(np.float32)

    # gather table: dinv * x, node-major rows in permuted slot order
    gtab = np.zeros((TS, NSLOT, DIN), np.float16)
    for t in range(TS):
        gtab[t, gslot] = (
            np.asarray(inputs[f"x{t}"]).astype(np.float32) * dinv[t, gslot][:, None]
        ).astype(np.float16)

    # feature-major per-core slice of the same table (self-loop add) in f32
    xselfT = np.empty((NCORE, TS, DIN, SPC), np.float32)
    for k in range(NCORE):
        sl = slice(k * SPC, (k + 1) * SPC)
        for t in range(TS):
            xselfT[k, t] = gtab[t, sl].astype(np.float32).T

    # dst-side dinv rows, replicated across partitions (DVE cannot
    # broadcast along the partition axis)
    dinv_myT = np.empty((NCORE, TS, 1, SPC), np.float32)
    for k in range(NCORE):
        dinv_myT[k, :, 0, :] = dinv[:, k * SPC : (k + 1) * SPC]
    dinv_myT = np.ascontiguousarray(np.broadcast_to(dinv_myT, (NCORE, TS, P, SPC)))

    # ---- edge cells: (core, window, chunk) with ragged per-(t,w,c) caps
    # (max over cores, so the SPMD instruction stream is core-uniform) ----
    ncell = NCORE * NW * CHN
    cells = []
    CBX = np.zeros((TS, NW, CHN), np.int32)  # blocks per cell
    for t in range(TS):
        src, dst = edges[t]
        gs, gd = gslot[src], gslot[dst]
        key = (gd // P) * CHN + gs // CHSZ  # (core*NW + w) * CHN + chunk
        srcloc = gs % CHSZ
        dstrel = gd % P
        # sort by (cell, srcloc) so each call's gather walks HBM in order
        o = np.lexsort((srcloc, key))
        key_s, srcloc_s, dstrel_s = key[o], srcloc[o], dstrel[o]
        cnt = np.bincount(key_s, minlength=ncell)
        CBX[t] = -(-cnt.reshape(NCORE, NW, CHN).max(axis=0) // P)
        starts = np.concatenate([[0], np.cumsum(cnt)[:-1]])
        cells.append((srcloc_s, dstrel_s, cnt, starts))

    # per-(t, g, c) call block counts and the max shapes for the DRAM arrays
    nblk_call = np.zeros((TS, NWG, CHN), np.int32)
    for t in range(TS):
        for g in range(NWG):
            nblk_call[t, g] = CBX[t, g * WGS : (g + 1) * WGS].sum(axis=0)
    max_call = int(nblk_call.max())          # blocks per gather call (max)
    max_wblk = int(CBX.sum(axis=2).max())    # blocks per window (max)
    ICOLS = max_call * P // 16
    DCOLS = int(nblk_call.sum(axis=2).max())  # blocks per (t,g) group (max)

    idx_arr = np.zeros((NCORE, TS, CHN, NWG, 16, ICOLS), np.int16)
    dst_arr = np.full((NCORE, TS, NWG, P, DCOLS), -1.0, np.float16)

    for t in range(TS):
        srcloc_s, dstrel_s, cnt, starts = cells[t]
        for k in range(NCORE):
            for g in range(NWG):
                dcol = 0
                for c in range(CHN):
                    parts = []
                    for wl in range(WGS):
                        w = g * WGS + wl
                        cell = (k * NW + w) * CHN + c
                        n = cnt[cell]
                        cap = CBX[t, w, c] * P
                        a = np.zeros(cap, np.int64)
                        a[:n] = srcloc_s[starts[cell] : starts[cell] + n]
                        parts.append(a)
                    flat = np.concatenate(parts)
                    assert flat.max(initial=0) < CHSZ
                    ncol = len(flat) // 16
                    idx_arr[k, t, c, g, :, :ncol] = (
                        flat.astype(np.int16).reshape(-1, 16).T
                    )
                # dst columns: per window, blocks ordered (c, b)
                for wl in range(WGS):
                    w = g * WGS + wl
                    for c in range(CHN):
                        cell = (k * NW + w) * CHN + c
                        n = cnt[cell]
                        cap = CBX[t, w, c] * P
                        a = np.full(cap, -1.0, np.float32)
                        a[:n] = dstrel_s[starts[cell] : starts[cell] + n]
                        nb = CBX[t, w, c]
                        dst_arr[k, t, g, :, dcol : dcol + nb] = (
                            a.reshape(nb, P).T.astype(np.float16)
                        )
                        dcol += nb

    idx_arr = np.ascontiguousarray(np.tile(idx_arr, (1, 1, 1, 1, 8, 1)))

    wts = dict(lin2_w=np.asarray(inputs["lin2_w"]).astype(np.float32))
    lin1 = np.asarray(inputs["lin1_w"]).astype(np.float32)
    for gname in "zrh":
        cw = np.asarray(inputs[f"convW_{gname}"]).astype(np.float32)
        lw = np.asarray(inputs[f"linW_{gname}"]).astype(np.float32)
        # fuse lin1 and conv into the gate projection: Z @ lin1 @ convW @ linW_top
        wts[f"Wg_{gname}"] = (lin1 @ cw @ lw[:DH]).astype(np.float16)
        wts[f"linWb_{gname}"] = lw[DH:].astype(np.float16)

    meta = dict(CBX=CBX, nblk_call=nblk_call, max_call=max_call,
                max_wblk=max_wblk, ICOLS=ICOLS, DCOLS=DCOLS)
    return dict(
        gtab=gtab, xselfT=xselfT, dinv_myT=dinv_myT,
        idx_arr=idx_arr, dst_arr=dst_arr, wts=wts, meta=meta,
    )


def _build(meta, ndev=NCORE):
    CBX = meta["CBX"]
    nblk_call = meta["nblk_call"]
    ICOLS = meta["ICOLS"]
    DCOLS = meta["DCOLS"]
    max_call = meta["max_call"]
    max_wblk = meta["max_wblk"]

    nc = bacc.Bacc("TRN2", target_bir_lowering=False, debug=False,
                   num_devices=ndev, num_swdge_queues=4,
                   dynamic_dma_scratch_size=32768)

    gtab_in = nc.dram_tensor("gtab", [TS, NSLOT, DIN], F16, kind="ExternalInput")
    xs_in = nc.dram_tensor("xselfT", [TS, DIN, SPC], F32, kind="ExternalInput")
    dim_in = nc.dram_tensor("dinv_myT", [TS, P, SPC], F32, kind="ExternalInput")
    idx_in = nc.dram_tensor("idx_arr", [TS, CHN, NWG, P, ICOLS], I16,
                            kind="ExternalInput")
    dst_in = nc.dram_tensor("dst_arr", [TS, NWG, P, DCOLS], F16,
                            kind="ExternalInput")
    Wg_in = {g: nc.dram_tensor(f"Wg_{g}", [DIN, DH], F16, kind="ExternalInput")
             for g in "zrh"}
    linWb_in = {g: nc.dram_tensor(f"linWb_{g}", [DH, DH], F16, kind="ExternalInput")
                for g in "zrh"}
    lin2_in = nc.dram_tensor("lin2_w", [DH, DOUT], F32, kind="ExternalInput")
    out_t = nc.dram_tensor("out", [1, DOUT], F32, kind="ExternalOutput")

    with tile.TileContext(nc) as tc:
        with (
            tc.tile_pool(name="const", bufs=1) as cpool,
            tc.tile_pool(name="hpool", bufs=1) as hpool,
            tc.tile_pool(name="pa", bufs=2) as pa,
            tc.tile_pool(name="gb", bufs=2) as gb,          # gather bufs
            tc.tile_pool(name="bc", bufs=3) as bcp,         # phase B/C tiles
            tc.tile_pool(name="ps", bufs=1, space="PSUM") as ps,
            tc.tile_pool(name="dram", bufs=1, space="DRAM") as dr,
        ):
            # constants
            Wg_sb = {}
            linWb_sb = {}
            for g in "zrh":
                Wg_sb[g] = cpool.tile([DIN, DH], F16, tag=f"wg{g}", name=f"wg{g}")
                nc.sync.dma_start(Wg_sb[g][:], Wg_in[g][:])
                linWb_sb[g] = cpool.tile([DH, DH], F16, tag=f"lb{g}", name=f"lb{g}")
                nc.sync.dma_start(linWb_sb[g][:], linWb_in[g][:])
            lin2_sb = cpool.tile([DH, 16], F32, tag="l2")
            nc.gpsimd.memset(lin2_sb[:], 0.0)
            nc.sync.dma_start(lin2_sb[:, :DOUT], lin2_in[:])

            iota_i = cpool.tile([P, P], I32, tag="ioi")
            nc.gpsimd.iota(iota_i[:], pattern=[[1, P]], base=0, channel_multiplier=0)
            iota_f = cpool.tile([P, P], F16, tag="iof")
            nc.vector.tensor_copy(iota_f[:], iota_i[:])

            H_sb = hpool.tile([DH, SPC], F16, tag="H")
            nc.gpsimd.memset(H_sb[:], 0.0)

            for t in range(TS):
                for g in range(NWG):
                    Gt = [None] * CHN
                    for c in range(CHN):
                        nidx = int(nblk_call[t, g, c]) * P
                        ix = gb.tile([P, ICOLS], I16, tag=f"ix{c}")
                        nc.sync.dma_start(ix[:, : nidx // 16],
                                          idx_in[t, c, g, :, : nidx // 16])
                        Gt[c] = gb.tile([P, max_call * P], F16, tag=f"G{c}",
                                        name=f"G{c}")
                        g3 = Gt[c][:, : nidx].rearrange("p (b q) -> p b q", q=P)
                        nc.gpsimd.dma_gather(
                            g3,
                            gtab_in[t, c * CHSZ : (c + 1) * CHSZ, :],
                            ix[:, : nidx // 16],
                            num_idxs=nidx,
                            num_idxs_reg=nidx,
                            elem_size=P,
                            single_packet=False,
                            queue_num=c,
                        )
                    gcols = int(nblk_call[t, g].sum())
                    dst_sb = gb.tile([P, DCOLS], F16, tag="dst")
                    nc.sync.dma_start(dst_sb[:, :gcols], dst_in[t, g, :, :gcols])
                    dinv_g = pa.tile([P, WGS * P], F32, tag="dim", bufs=2)
                    nc.sync.dma_start(
                        dinv_g[:], dim_in[t, :, g * WGS * P : (g + 1) * WGS * P]
                    )

                    y_ps = [
                        ps.tile([P, 512], F32, tag="Y", name="Y0", bufs=4),
                        ps.tile([P, 512], F32, tag="Y", name="Y1", bufs=4),
                    ]
                    goff = [0] * CHN  # per-chunk block offset in Gt
                    dcol = 0          # column offset in dst_sb
                    for wl in range(WGS):
                        w = g * WGS + wl
                        ycol = y_ps[wl // 4][:, (wl % 4) * P : (wl % 4 + 1) * P]
                        nbw = int(CBX[t, w].sum())  # blocks for this window
                        # selection matrices for this window's blocks in one op
                        M01 = bcp.tile([P, max_wblk * P], F16, tag="m01")
                        m3 = M01[:, : nbw * P].rearrange("p (b q) -> p b q", b=nbw)
                        i0 = iota_f[:].rearrange("p (b q) -> p b q", b=1)
                        i1 = dst_sb[:, dcol : dcol + nbw][:, :, None]
                        a0, a1 = broadcast_tensor_aps(i0, i1)
                        nc.vector.tensor_tensor(out=m3, in0=a0, in1=a1,
                                                op=mybir.AluOpType.is_equal)
                        dcol += nbw
                        # aggregate feature-major: Z^T[f, dst] += G^T M01
                        j = 0
                        for c in range(CHN):
                            nb = int(CBX[t, w, c])
                            for b in range(nb):
                                nc.tensor.matmul(
                                    ycol,
                                    lhsT=Gt[c][:, (goff[c] + b) * P : (goff[c] + b + 1) * P],
                                    rhs=M01[:, (j + b) * P : (j + b + 1) * P],
                                    start=(j + b == 0),
                                    stop=(j + b == nbw - 1),
                                )
                            goff[c] += nb
                            j += nb
                    # ---- GRU in 2 batches: windows [0:4) and [4:7) ----
                    for bi, (w0, nwb) in enumerate(((0, 4), (4, 3))):
                        W = nwb * P
                        n0 = (g * WGS + w0) * P  # node-column base
                        nsl = slice(n0, n0 + W)
                        Hsl = H_sb[:, nsl]
                        drow = dinv_g[:, w0 * P : w0 * P + W]
                        # self-loop rows (feature-major) + dst-side dinv
                        xself_sb = bcp.tile([P, 512], F32, tag="xself")
                        nc.sync.dma_start(xself_sb[:, :W], xs_in[t, :, nsl])
                        y0_sb = bcp.tile([P, 512], F32, tag="y0")
                        nc.vector.tensor_add(y0_sb[:, :W], y_ps[bi][:, :W],
                                             xself_sb[:, :W])
                        Zt_sb = bcp.tile([P, 512], F16, tag="Zt")
                        nc.vector.tensor_tensor(out=Zt_sb[:, :W], in0=y0_sb[:, :W],
                                                in1=drow, op=mybir.AluOpType.mult)
                        # gates (lin1+conv fused into Wg on host)
                        ZR = {}
                        for gname in "zr":
                            A_ps = ps.tile([P, 512], F32, tag="pc",
                                           name=f"Aps{gname}", bufs=4)
                            nc.tensor.matmul(A_ps[:, :W], lhsT=Wg_sb[gname][:],
                                             rhs=Zt_sb[:, :W], start=True, stop=False)
                            nc.tensor.matmul(A_ps[:, :W], lhsT=linWb_sb[gname][:],
                                             rhs=Hsl, start=False, stop=True)
                            ZR[gname] = bcp.tile([P, 512], F16, tag=gname.upper(),
                                                 name=gname.upper())
                            nc.scalar.activation(ZR[gname][:, :W], A_ps[:, :W],
                                                 mybir.ActivationFunctionType.Sigmoid)
                        HR = bcp.tile([P, 512], F16, tag="HR")
                        nc.vector.tensor_mul(HR[:, :W], Hsl, ZR["r"][:, :W])
                        A_ps = ps.tile([P, 512], F32, tag="pc", name="Apsh", bufs=4)
                        nc.tensor.matmul(A_ps[:, :W], lhsT=Wg_sb["h"][:],
                                         rhs=Zt_sb[:, :W], start=True, stop=False)
                        nc.tensor.matmul(A_ps[:, :W], lhsT=linWb_sb["h"][:],
                                         rhs=HR[:, :W], start=False, stop=True)
                        Ht = bcp.tile([P, 512], F16, tag="Ht")
                        nc.scalar.activation(Ht[:, :W], A_ps[:, :W],
                                             mybir.ActivationFunctionType.Tanh)
                        # H = Ht + Z*(H - Ht)
                        Hd = bcp.tile([P, 512], F16, tag="Hd")
                        nc.vector.tensor_sub(Hd[:, :W], Hsl, Ht[:, :W])
                        nc.vector.tensor_mul(Hd[:, :W], ZR["z"][:, :W], Hd[:, :W])
                        nc.vector.tensor_add(Hsl, Ht[:, :W], Hd[:, :W])

            # ---- final: masked max pool + AllReduce + projection ----
            nc.gpsimd.memset(H_sb[:, REAL_PC:SPC], -10000.0)
            hmax = cpool.tile([P, 1], F32, tag="hmax")
            nc.vector.reduce_max(hmax[:], H_sb[:], axis=mybir.AxisListType.X)
            cc_in = dr.tile([P, 1], F32)
            cc_out = dr.tile([P, 1], F32)
            nc.sync.dma_start(cc_in[:], hmax[:])
            if ndev > 1:
                nc.gpsimd.collective_compute(
                    "AllReduce",
                    mybir.AluOpType.max,
                    replica_groups=[list(range(NCORE))],
                    ins=[cc_in.opt()],
                    outs=[cc_out.opt()],
                )
            else:
                nc.gpsimd.dma_start(cc_out[:], cc_in[:])
            hg = cpool.tile([P, 1], F32, tag="hg")
            nc.sync.dma_start(hg[:], cc_out[:])
            o_ps = ps.tile([1, 16], F32, tag="pc", bufs=4)
            nc.tensor.matmul(o_ps[:, :16], lhsT=hg[:], rhs=lin2_sb[:],
                             start=True, stop=True)
            o_sb = cpool.tile([1, 16], F32, tag="osb")
            nc.vector.tensor_copy(o_sb[:], o_ps[:])
            nc.sync.dma_start(out_t[:], o_sb[:, :DOUT])

    nc.compile()
    return nc


def _make_in_maps(pre):
    in_maps = []
    for k in range(NCORE):
        in_maps.append(
            dict(
                gtab=pre["gtab"],
                xselfT=np.ascontiguousarray(pre["xselfT"][k]),
                dinv_myT=np.ascontiguousarray(pre["dinv_myT"][k]),
                idx_arr=np.ascontiguousarray(pre["idx_arr"][k]),
                dst_arr=np.ascontiguousarray(pre["dst_arr"][k]),
                lin2_w=pre["wts"]["lin2_w"],
                **{f"Wg_{g}": pre["wts"][f"Wg_{g}"] for g in "zrh"},
                **{f"linWb_{g}": pre["wts"][f"linWb_{g}"] for g in "zrh"},
            )
        )
    return in_maps


def _postprocess(res, pre):
    return res.results[0]["out"].astype(np.float32)


def kernel(**inputs) -> np.ndarray:
    pre = _preprocess(inputs)
    nc = _build(pre["meta"])
    in_maps = _make_in_maps(pre)
    res = run_bass_kernel_spmd(nc, in_maps, core_ids=list(range(NCORE)))
    return _postprocess(res, pre)


if __name__ == "__main__":
    d = dict(np.load("/root/problem/inputs_cache.npz"))
    out = kernel(**d)
    print("kernel out:", out)


# revision 35
# speedup vs baseline: 1.4839x; 1.4839x over previous
"""TGCN (3-step GRU over GCN message passing) on 8 Trainium2 NeuronCores.

Strategy (dst-sharded message passing):
- Host relabels nodes (max-pool over nodes is permutation invariant) with a
  degree-balanced LPT assignment into 8 cores x 98 windows x 128 slots.
- Associativity: gcn(x@lin1) = (Anorm @ (dinv*x)) @ lin1 ... with lin1 and
  the conv weight folded into the gate projections on the host. The gather
  table is therefore dinv*x — pure host data, staged node-major in 4
  contiguous chunks of 25088 rows (int16-indexable). No phase A on device.
- Per (7-window group, src chunk): dma_gather (rotating over 4 SWDGE queues,
  with a deep descriptor-ring carveout so rings drain in parallel with
  generation) fetches per-edge source rows; 0/1 selection matrices built
  with iota+is_equal route each 128-edge block into the group's PSUM
  accumulator via the PE (scatter-add as matmul, gathered block stationary
  so the accumulator is feature-major - no transposes anywhere).
- Self-loops skip the gather: the feature-major dinv*x slice of the core's
  own nodes is DMA'd and added on the vector engine.
- GRU gates as 512-wide fp16 matmuls feature-major, conv+lin1 fused into
  the gate weights (biases are all zero); H stays resident in SBUF.
- Final: per-feature max over the core's nodes, AllReduce-max across cores,
  then the 128x10 output projection (identical on every core).
"""
import sys

sys.path.insert(0, "/opt/trn_rl_repo")

import numpy as np

import concourse.bass as bass
import concourse.mybir as mybir
import concourse.tile as tile
import concourse.bacc as bacc
from concourse.bass import broadcast_tensor_aps
from concourse.bass_utils import run_bass_kernel_spmd

F16 = mybir.dt.float16
F32 = mybir.dt.float32
I16 = mybir.dt.int16
I32 = mybir.dt.int32

N = 100000
E = 1600000
DIN = 128
DH = 128
DOUT = 10
P = 128
NCORE = 8
NW = 98               # windows (128-slot dst tiles) per core
SPC = NW * P          # 12544 slots per core
NSLOT = NCORE * SPC   # 100352
NT = NSLOT // P       # 784 global tiles
REAL_PC = 12500       # real nodes per core; pads at slots [12500, 12544)
CHN = 4               # source chunks: contiguous tile ranges of 196 tiles
CHTILES = NT // CHN   # 196
CHSZ = CHTILES * P    # 25088 rows per chunk (< 32768: int16-safe)
WGS = 7               # windows per gather group
NWG = NW // WGS       # 14 groups
TS = 3


def _preprocess(inputs):
    """Numpy-only host prep: node relabeling, edge sharding, input staging."""
    for b in ("lin1_b", "convb_z", "convb_r", "convb_h",
              "linb_z", "linb_r", "linb_h", "lin2_b"):
        assert np.abs(np.asarray(inputs[b])).max() == 0.0, f"{b} nonzero"

    import heapq

    edges = [np.asarray(inputs[f"edge{t}"]).astype(np.int64) for t in range(TS)]
    deg3 = np.zeros(N, np.int64)
    for t in range(TS):
        deg3 += np.bincount(edges[t][1], minlength=N)
    w_nodes = deg3 + 3

    order = np.argsort(-w_nodes, kind="stable")
    nbins = NCORE * NW
    cap = np.full(nbins, P, np.int32)
    cap[NW - 1 :: NW] = REAL_PC - (NW - 1) * P  # 84 real slots in last window
    heap = [(0, b) for b in range(nbins)]
    heapq.heapify(heap)
    bin_count = np.zeros(nbins, np.int32)
    bin_load = np.zeros(nbins, np.int64)
    assign_bin = np.empty(N, np.int32)
    slot_in_bin = np.empty(N, np.int32)
    for n in order:
        load, b = heapq.heappop(heap)
        assign_bin[n] = b
        slot_in_bin[n] = bin_count[b]
        bin_count[b] += 1
        bin_load[b] += w_nodes[n]
        if bin_count[b] < cap[b]:
            heapq.heappush(heap, (bin_load[b], b))
    core_of = assign_bin // NW
    w_of = assign_bin % NW
    gslot = (core_of * SPC + w_of * P + slot_in_bin).astype(np.int64)

    # degrees (with +1 self loop); pads get 1.0
    dinv = np.empty((TS, NSLOT), np.float32)
    for t in range(TS):
        dd = np.bincount(gslot[edges[t][1]], minlength=NSLOT).astype(np.float64)
        dd += 1.0  # self loops (pads harmlessly get deg 1: their rows are 0)
        dinv[t] = (1.0 / np.sqrt(dd)).astype(np.float32)

    # gather table: dinv * x, node-major rows in permuted slot order
    gtab = np.zeros((TS, NSLOT, DIN), np.float16)
    for t in range(TS):
        gtab[t, gslot] = (
            np.asarray(inputs[f"x{t}"]).astype(np.float32) * dinv[t, gslot][:, None]
        ).astype(np.float16)

    # feature-major per-core slice of the same table (self-loop add) in f32
    xselfT = np.empty((NCORE, TS, DIN, SPC), np.float32)
    for k in range(NCORE):
        sl = slice(k * SPC, (k + 1) * SPC)
        for t in range(TS):
            xselfT[k, t] = gtab[t, sl].astype(np.float32).T

    # dst-side dinv rows, replicated across partitions (DVE cannot
    # broadcast along the partition axis)
    dinv_myT = np.empty((NCORE, TS, 1, SPC), np.float32)
    for k in range(NCORE):
        dinv_myT[k, :, 0, :] = dinv[:, k * SPC : (k + 1) * SPC]
    dinv_myT = np.ascontiguousarray(np.broadcast_to(dinv_myT, (NCORE, TS, P, SPC)))

    # ---- edge cells: (core, window, chunk). Each (group, chunk) gather call
    # packs its 7 cells at offsets sized by the MAX count over cores, so the
    # SPMD instruction stream is core-uniform while pad descriptors shrink.
    ncell = NCORE * NW * CHN
    cells = []
    cmax = np.zeros((TS, NW, CHN), np.int64)  # max cell count over cores
    for t in range(TS):
        src, dst = edges[t]
        gs, gd = gslot[src], gslot[dst]
        key = (gd // P) * CHN + gs // CHSZ  # (core*NW + w) * CHN + chunk
        srcloc = gs % CHSZ
        dstrel = gd % P
        # sort by (cell, srcloc) so each call's gather walks HBM in order
        o = np.lexsort((srcloc, key))
        key_s, srcloc_s, dstrel_s = key[o], srcloc[o], dstrel[o]
        cnt = np.bincount(key_s, minlength=ncell)
        cmax[t] = cnt.reshape(NCORE, NW, CHN).max(axis=0)
        starts = np.concatenate([[0], np.cumsum(cnt)[:-1]])
        cells.append((srcloc_s, dstrel_s, cnt, starts))

    # call layout: per (t,g,c), cell wl at offset OFF = cumsum of cmax
    OFF = np.zeros((TS, NWG, CHN, WGS + 1), np.int64)
    for t in range(TS):
        for g in range(NWG):
            for c in range(CHN):
                OFF[t, g, c, 1:] = np.cumsum(
                    cmax[t, g * WGS : (g + 1) * WGS, c]
                )
    NBC = (-(-OFF[:, :, :, WGS] // P)).astype(np.int64)  # blocks per call
    # per-(t,g,wl,c) block span of the cell within its call
    BLK0 = OFF[:, :, :, :WGS] // P                       # [TS,NWG,CHN,WGS]
    BLK1 = -(-(OFF[:, :, :, :WGS] + cmax.reshape(
        TS, NWG, WGS, CHN).transpose(0, 1, 3, 2)) // P)
    max_call = int(NBC.max())
    ICOLS = max_call * P // 16
    SPAN = (BLK1 - BLK0).astype(np.int64)
    DCOLS = int(SPAN.sum(axis=(2, 3)).max())  # dst cols per (t,g) group (max)
    max_wblk = int(SPAN.sum(axis=2).max())    # blocks per window (max)

    idx_arr = np.zeros((NCORE, TS, CHN, NWG, 16, ICOLS), np.int16)
    dst_arr = np.full((NCORE, TS, NWG, P, DCOLS), -1.0, np.float16)

    for t in range(TS):
        srcloc_s, dstrel_s, cnt, starts = cells[t]
        for k in range(NCORE):
            for g in range(NWG):
                # build per-call position-axis idx/dst vectors
                call_dst = {}
                for c in range(CHN):
                    L = int(NBC[t, g, c]) * P
                    ci = np.zeros(L, np.int64)
                    cd = np.full(L, -1.0, np.float32)
                    for wl in range(WGS):
                        w = g * WGS + wl
                        cell = (k * NW + w) * CHN + c
                        n = int(cnt[cell])
                        o0 = int(OFF[t, g, c, wl])
                        ci[o0 : o0 + n] = srcloc_s[
                            starts[cell] : starts[cell] + n]
                        cd[o0 : o0 + n] = dstrel_s[
                            starts[cell] : starts[cell] + n]
                    assert ci.max(initial=0) < CHSZ
                    idx_arr[k, t, c, g, :, : L // 16] = (
                        ci.astype(np.int16).reshape(-1, 16).T
                    )
                    call_dst[c] = cd
                # dst columns: per (wl, c): span blocks sliced from call_dst,
                # masked to this window's position range
                dcol = 0
                for wl in range(WGS):
                    for c in range(CHN):
                        o0 = int(OFF[t, g, c, wl])
                        o1 = int(OFF[t, g, c, wl + 1])
                        b0 = int(BLK0[t, g, c, wl])
                        b1 = int(BLK1[t, g, c, wl])
                        sl = call_dst[c][b0 * P : b1 * P].copy()
                        pos = np.arange(b0 * P, b1 * P)
                        sl[(pos < o0) | (pos >= o1)] = -1.0
                        dst_arr[k, t, g, :, dcol : dcol + (b1 - b0)] = (
                            sl.reshape(b1 - b0, P).T.astype(np.float16)
                        )
                        dcol += b1 - b0

    idx_arr = np.ascontiguousarray(np.tile(idx_arr, (1, 1, 1, 1, 8, 1)))

    wts = dict(lin2_w=np.asarray(inputs["lin2_w"]).astype(np.float32))
    lin1 = np.asarray(inputs["lin1_w"]).astype(np.float32)
    for gname in "zrh":
        cw = np.asarray(inputs[f"convW_{gname}"]).astype(np.float32)
        lw = np.asarray(inputs[f"linW_{gname}"]).astype(np.float32)
        # fuse lin1 and conv into the gate projection: Z @ lin1 @ convW @ linW_top
        wts[f"Wg_{gname}"] = (lin1 @ cw @ lw[:DH]).astype(np.float16)
        wts[f"linWb_{gname}"] = lw[DH:].astype(np.float16)

    meta = dict(NBC=NBC, SPAN=SPAN, BLK0=BLK0, max_call=max_call,
                max_wblk=max_wblk, ICOLS=ICOLS, DCOLS=DCOLS)
    return dict(
        gtab=gtab, xselfT=xselfT, dinv_myT=dinv_myT,
        idx_arr=idx_arr, dst_arr=dst_arr, wts=wts, meta=meta,
    )


def _build(meta, ndev=NCORE):
    NBC = meta["NBC"]
    SPAN = meta["SPAN"]
    BLK0 = meta["BLK0"]
    ICOLS = meta["ICOLS"]
    DCOLS = meta["DCOLS"]
    max_call = meta["max_call"]
    max_wblk = meta["max_wblk"]

    nc = bacc.Bacc("TRN2", target_bir_lowering=False, debug=False,
                   num_devices=ndev, num_swdge_queues=4,
                   dynamic_dma_scratch_size=32768)

    gtab_in = nc.dram_tensor("gtab", [TS, NSLOT, DIN], F16, kind="ExternalInput")
    xs_in = nc.dram_tensor("xselfT", [TS, DIN, SPC], F32, kind="ExternalInput")
    dim_in = nc.dram_tensor("dinv_myT", [TS, P, SPC], F32, kind="ExternalInput")
    idx_in = nc.dram_tensor("idx_arr", [TS, CHN, NWG, P, ICOLS], I16,
                            kind="ExternalInput")
    dst_in = nc.dram_tensor("dst_arr", [TS, NWG, P, DCOLS], F16,
                            kind="ExternalInput")
    Wg_in = {g: nc.dram_tensor(f"Wg_{g}", [DIN, DH], F16, kind="ExternalInput")
             for g in "zrh"}
    linWb_in = {g: nc.dram_tensor(f"linWb_{g}", [DH, DH], F16, kind="ExternalInput")
                for g in "zrh"}
    lin2_in = nc.dram_tensor("lin2_w", [DH, DOUT], F32, kind="ExternalInput")
    out_t = nc.dram_tensor("out", [1, DOUT], F32, kind="ExternalOutput")

    with tile.TileContext(nc) as tc:
        with (
            tc.tile_pool(name="const", bufs=1) as cpool,
            tc.tile_pool(name="hpool", bufs=1) as hpool,
            tc.tile_pool(name="pa", bufs=2) as pa,
            tc.tile_pool(name="gb", bufs=2) as gb,          # gather bufs
            tc.tile_pool(name="bc", bufs=3) as bcp,         # phase B/C tiles
            tc.tile_pool(name="ps", bufs=1, space="PSUM") as ps,
            tc.tile_pool(name="dram", bufs=1, space="DRAM") as dr,
        ):
            # constants
            Wg_sb = {}
            linWb_sb = {}
            for g in "zrh":
                Wg_sb[g] = cpool.tile([DIN, DH], F16, tag=f"wg{g}", name=f"wg{g}")
                nc.sync.dma_start(Wg_sb[g][:], Wg_in[g][:])
                linWb_sb[g] = cpool.tile([DH, DH], F16, tag=f"lb{g}", name=f"lb{g}")
                nc.sync.dma_start(linWb_sb[g][:], linWb_in[g][:])
            lin2_sb = cpool.tile([DH, 16], F32, tag="l2")
            nc.gpsimd.memset(lin2_sb[:], 0.0)
            nc.sync.dma_start(lin2_sb[:, :DOUT], lin2_in[:])

            iota_i = cpool.tile([P, P], I32, tag="ioi")
            nc.gpsimd.iota(iota_i[:], pattern=[[1, P]], base=0, channel_multiplier=0)
            iota_f = cpool.tile([P, P], F16, tag="iof")
            nc.vector.tensor_copy(iota_f[:], iota_i[:])

            H_sb = hpool.tile([DH, SPC], F16, tag="H")
            nc.gpsimd.memset(H_sb[:], 0.0)

            for t in range(TS):
                for g in range(NWG):
                    Gt = [None] * CHN
                    for c in range(CHN):
                        nidx = int(NBC[t, g, c]) * P
                        ix = gb.tile([P, ICOLS], I16, tag=f"ix{c}")
                        nc.sync.dma_start(ix[:, : nidx // 16],
                                          idx_in[t, c, g, :, : nidx // 16])
                        Gt[c] = gb.tile([P, max_call * P], F16, tag=f"G{c}",
                                        name=f"G{c}")
                        g3 = Gt[c][:, : nidx].rearrange("p (b q) -> p b q", q=P)
                        nc.gpsimd.dma_gather(
                            g3,
                            gtab_in[t, c * CHSZ : (c + 1) * CHSZ, :],
                            ix[:, : nidx // 16],
                            num_idxs=nidx,
                            num_idxs_reg=nidx,
                            elem_size=P,
                            single_packet=False,
                            queue_num=c,
                        )
                    gcols = int(SPAN[t, g].sum())
                    dst_sb = gb.tile([P, DCOLS], F16, tag="dst")
                    nc.sync.dma_start(dst_sb[:, :gcols], dst_in[t, g, :, :gcols])
                    dinv_g = pa.tile([P, WGS * P], F32, tag="dim", bufs=2)
                    nc.sync.dma_start(
                        dinv_g[:], dim_in[t, :, g * WGS * P : (g + 1) * WGS * P]
                    )

                    y_ps = [
                        ps.tile([P, 512], F32, tag="Y", name="Y0", bufs=4),
                        ps.tile([P, 512], F32, tag="Y", name="Y1", bufs=4),
                    ]
                    dcol = 0  # column offset in dst_sb
                    for wl in range(WGS):
                        ycol = y_ps[wl // 4][:, (wl % 4) * P : (wl % 4 + 1) * P]
                        nbw = int(SPAN[t, g, :, wl].sum())
                        # selection matrices for this window's blocks in one op
                        M01 = bcp.tile([P, max_wblk * P], F16, tag="m01")
                        m3 = M01[:, : nbw * P].rearrange("p (b q) -> p b q", b=nbw)
                        i0 = iota_f[:].rearrange("p (b q) -> p b q", b=1)
                        i1 = dst_sb[:, dcol : dcol + nbw][:, :, None]
                        a0, a1 = broadcast_tensor_aps(i0, i1)
                        nc.vector.tensor_tensor(out=m3, in0=a0, in1=a1,
                                                op=mybir.AluOpType.is_equal)
                        dcol += nbw
                        # aggregate feature-major: Z^T[f, dst] += G^T M01
                        j = 0
                        for c in range(CHN):
                            b0 = int(BLK0[t, g, c, wl])
                            for b in range(int(SPAN[t, g, c, wl])):
                                nc.tensor.matmul(
                                    ycol,
                                    lhsT=Gt[c][:, (b0 + b) * P : (b0 + b + 1) * P],
                                    rhs=M01[:, (j + b) * P : (j + b + 1) * P],
                                    start=(j + b == 0),
                                    stop=(j + b == nbw - 1),
                                )
                            j += int(SPAN[t, g, c, wl])
                    # ---- GRU in 2 batches: windows [0:4) and [4:7) ----
                    for bi, (w0, nwb) in enumerate(((0, 4), (4, 3))):
                        W = nwb * P
                        n0 = (g * WGS + w0) * P  # node-column base
                        nsl = slice(n0, n0 + W)
                        Hsl = H_sb[:, nsl]
                        drow = dinv_g[:, w0 * P : w0 * P + W]
                        # self-loop rows (feature-major) + dst-side dinv
                        xself_sb = bcp.tile([P, 512], F32, tag="xself")
                        nc.sync.dma_start(xself_sb[:, :W], xs_in[t, :, nsl])
                        y0_sb = bcp.tile([P, 512], F32, tag="y0")
                        nc.vector.tensor_add(y0_sb[:, :W], y_ps[bi][:, :W],
                                             xself_sb[:, :W])
                        Zt_sb = bcp.tile([P, 512], F16, tag="Zt")
                        nc.vector.tensor_tensor(out=Zt_sb[:, :W], in0=y0_sb[:, :W],
                                                in1=drow, op=mybir.AluOpType.mult)
                        # gates (lin1+conv fused into Wg on host)
                        ZR = {}
                        for gname in "zr":
                            A_ps = ps.tile([P, 512], F32, tag="pc",
                                           name=f"Aps{gname}", bufs=4)
                            nc.tensor.matmul(A_ps[:, :W], lhsT=Wg_sb[gname][:],
                                             rhs=Zt_sb[:, :W], start=True, stop=False)
                            nc.tensor.matmul(A_ps[:, :W], lhsT=linWb_sb[gname][:],
                                             rhs=Hsl, start=False, stop=True)
                            ZR[gname] = bcp.tile([P, 512], F16, tag=gname.upper(),
                                                 name=gname.upper())
                            nc.scalar.activation(ZR[gname][:, :W], A_ps[:, :W],
                                                 mybir.ActivationFunctionType.Sigmoid)
                        HR = bcp.tile([P, 512], F16, tag="HR")
                        nc.vector.tensor_mul(HR[:, :W], Hsl, ZR["r"][:, :W])
                        A_ps = ps.tile([P, 512], F32, tag="pc", name="Apsh", bufs=4)
                        nc.tensor.matmul(A_ps[:, :W], lhsT=Wg_sb["h"][:],
                                         rhs=Zt_sb[:, :W], start=True, stop=False)
                        nc.tensor.matmul(A_ps[:, :W], lhsT=linWb_sb["h"][:],
                                         rhs=HR[:, :W], start=False, stop=True)
                        Ht = bcp.tile([P, 512], F16, tag="Ht")
                        nc.scalar.activation(Ht[:, :W], A_ps[:, :W],
                                             mybir.ActivationFunctionType.Tanh)
                        # H = Ht + Z*(H - Ht)
                        Hd = bcp.tile([P, 512], F16, tag="Hd")
                        nc.vector.tensor_sub(Hd[:, :W], Hsl, Ht[:, :W])
                        nc.vector.tensor_mul(Hd[:, :W], ZR["z"][:, :W], Hd[:, :W])
                        nc.vector.tensor_add(Hsl, Ht[:, :W], Hd[:, :W])

            # ---- final: masked max pool + AllReduce + projection ----
            nc.gpsimd.memset(H_sb[:, REAL_PC:SPC], -10000.0)
            hmax = cpool.tile([P, 1], F32, tag="hmax")
            nc.vector.reduce_max(hmax[:], H_sb[:], axis=mybir.AxisListType.X)
            cc_in = dr.tile([P, 1], F32)
            cc_out = dr.tile([P, 1], F32)
            nc.sync.dma_start(cc_in[:], hmax[:])
            if ndev > 1:
                nc.gpsimd.collective_compute(
                    "AllReduce",
                    mybir.AluOpType.max,
                    replica_groups=[list(range(NCORE))],
                    ins=[cc_in.opt()],
                    outs=[cc_out.opt()],
                )
            else:
                nc.gpsimd.dma_start(cc_out[:], cc_in[:])
            hg = cpool.tile([P, 1], F32, tag="hg")
            nc.sync.dma_start(hg[:], cc_out[:])
            o_ps = ps.tile([1, 16], F32, tag="pc", bufs=4)
            nc.tensor.matmul(o_ps[:, :16], lhsT=hg[:], rhs=lin2_sb[:],
                             start=True, stop=True)
            o_sb = cpool.tile([1, 16], F32, tag="osb")
            nc.vector.tensor_copy(o_sb[:], o_ps[:])
            nc.sync.dma_start(out_t[:], o_sb[:, :DOUT])

    nc.compile()
    return nc


def _make_in_maps(pre):
    in_maps = []
    for k in range(NCORE):
        in_maps.append(
            dict(
                gtab=pre["gtab"],
                xselfT=np.ascontiguousarray(pre["xselfT"][k]),
                dinv_myT=np.ascontiguousarray(pre["dinv_myT"][k]),
                idx_arr=np.ascontiguousarray(pre["idx_arr"][k]),
                dst_arr=np.ascontiguousarray(pre["dst_arr"][k]),
                lin2_w=pre["wts"]["lin2_w"],
                **{f"Wg_{g}": pre["wts"][f"Wg_{g}"] for g in "zrh"},
                **{f"linWb_{g}": pre["wts"][f"linWb_{g}"] for g in "zrh"},
            )
        )
    return in_maps


def _postprocess(res, pre):
    return res.results[0]["out"].astype(np.float32)


def kernel(**inputs) -> np.ndarray:
    pre = _preprocess(inputs)
    nc = _build(pre["meta"])
    in_maps = _make_in_maps(pre)
    res = run_bass_kernel_spmd(nc, in_maps, core_ids=list(range(NCORE)))
    return _postprocess(res, pre)


if __name__ == "__main__":
    d = dict(np.load("/root/problem/inputs_cache.npz"))
    out = kernel(**d)
    print("kernel out:", out)
